# revision 22
# baseline (speedup 1.0000x reference)
"""Trainium2 Bass kernel for CarbonAwareLSTM.

B=64, T=4096, F=64, U=128. Keras LSTM (gate order i,f,c,o), returns last
hidden state h_T [B, U].

Strategy (data-parallel over batch, 8 cores x 8 rows). The recurrence is
latency-bound (per-step serial chain across engines, ~1.85 us/step on
HW), so the design minimizes instructions on the per-step critical
cycle:

- Input projection z_seed = [K; bias]^T @ [x; 1] (bf16) is matmul'd
  directly into the per-step PSUM z tiles (no xw SBUF buffer, no
  evacuation instructions). The 4 seed matmuls for step t+1 are emitted
  right after step t's W matmuls so they ride PE idle gaps; bf16
  stationaries keep their Ldweights cost negligible (fp32 seeds
  measured +575 ns/step on HW).
- PSUM accumulation on HW only tolerates ONE open
  start->accumulate->stop group per 2KB bank (a start=True corrupts any
  other open group in its bank), so gate g of step-parity p owns bank
  4p+g of a single [U, 8, 512] PSUM tile: per bank, seed(t) -> W(t) ->
  seed(t+2) is strictly sequential. Wide multi-step seed groups also
  corrupt earlier columns and cannot be used.
- Per step: 4 accumulating W matmuls (bf16) onto the seeded PSUM; ONE
  sigmoid over all 4 gate blocks [U, 4, B] (the g gate's z is
  pre-scaled by 2 host-side so tanh(z) = 2*sigmoid(2z) - 1); DVE:
  v = (s_g - 0.5)*i then c = 2v + fc (scalar_tensor_tensor, two fused
  ops replace three); Pool (gpsimd) computes fc = f*c_old off the
  critical path; ACT tanh(c); DVE h = o*tanh(c) written directly as
  bf16 (stationary operand of the next step's W matmuls).
- c is double-buffered in SBUF (ping-pong) so Pool's read of c_old
  never serializes against the c_new write; h/th/sg all stay in SBUF
  (DVE pays 2x access latency on PSUM operands; ACT's PSUM preference
  never wins once its consumers' penalties are counted).
"""

import sys

sys.path.insert(0, "/opt/trn_rl_repo")

from contextlib import ExitStack

import numpy as np

import concourse.bacc as bacc
import concourse.bass as bass
import concourse.tile as tile
from concourse import mybir
from concourse.bass_utils import run_bass_kernel_spmd

B_TOTAL = 64
T_FULL = 4096
F = 64
U = 128
N_CORES = 8
B = B_TOTAL // N_CORES  # batch rows per core

F32 = mybir.dt.float32
BF16 = mybir.dt.bfloat16
AF = mybir.ActivationFunctionType
ALU = mybir.AluOpType

# gate block order used on device: [i, f, o, g]; reference order is [i, f, g, o]
GATE_PERM = [0, 1, 3, 2]
G_I, G_F, G_O, G_G = 0, 1, 2, 3


def build_nc(T: int, CH: int = 512, fc_pool: bool = True, n_loop: int = 1,
             tvar: str = "base", **_compat) -> bass.Bass:
    """Build the single-core Bass program (run SPMD on 8 cores).

    CH = timesteps per DMA chunk. fc_pool: compute fc = f*c_old on the
    Pool (gpsimd) engine instead of DVE. n_loop > 1 repeats the whole
    recurrence in a hardware loop (timing builds only). tvar selects
    timing-only structural variants (numerically wrong except "base"):
      noseed       - no seed matmuls (W-mm g0 start=True)
      sigma_contig - sigma reads a contiguous single-bank window
      onegroup     - v1-style single-bank z group, no seeds, contig sigma
    """
    assert T % CH == 0
    n_chunks = T // CH

    nc = bacc.Bacc(None, target_bir_lowering=False, debug=False)

    # xT: [F+1, B, T] with a trailing ones row (bias via contraction)
    xT_d = nc.dram_tensor("xT", [F + 1, B * T], BF16, kind="ExternalInput")
    w_d = nc.dram_tensor("w", [U, 4 * U], F32, kind="ExternalInput")
    kb_d = nc.dram_tensor("kb", [F + 1, 4 * U], BF16, kind="ExternalInput")
    out_d = nc.dram_tensor("hT_out", [U, B], BF16, kind="ExternalOutput")

    with tile.TileContext(nc) as tc, ExitStack() as ctx:
        singles = ctx.enter_context(tc.tile_pool(name="singles", bufs=1))
        xsb_pool = ctx.enter_context(tc.tile_pool(name="xsb", bufs=2))
        psZ = ctx.enter_context(tc.tile_pool(name="psZ", bufs=1, space="PSUM"))
        gates = ctx.enter_context(tc.tile_pool(name="gates", bufs=2))

        W_f32 = singles.tile([U, 4 * U], F32)
        nc.sync.dma_start(W_f32, w_d[:])
        W_sb = singles.tile([U, 4 * U], BF16)
        nc.vector.tensor_copy(W_sb, W_f32)
        Kb_sb = singles.tile([F + 1, 4 * U], BF16)
        nc.sync.dma_start(Kb_sb, kb_d[:])

        hT = singles.tile([U, B], BF16, tag="hT", name="hT")
        nc.vector.memset(hT, 0.0)
        cbuf = [
            singles.tile([U, B], F32, tag=f"c{p}", name=f"c{p}") for p in range(2)
        ]
        nc.vector.memset(cbuf[0], 0.0)

        xT_view = xT_d[:].rearrange("f (b t) -> f b t", b=B)

        # chunk-level SBUF x buffers (DMA'd one chunk ahead)
        xsb = []
        for k in range(min(n_chunks, 2)):
            t_ = xsb_pool.tile([F + 1, B, CH], BF16, tag="xsb", name=f"xsb{k}")
            xsb.append(t_)
        nc.sync.dma_start(xsb[0], xT_view[:, :, bass.ds(0, CH)])

        # z PSUM: one tile spanning all 8 banks; gate g of step-parity p
        # owns bank 4p+g (cols 0:B). Hardware PSUM accumulation requires
        # per-bank groups to be small and strictly sequential (a
        # start=True corrupts any other open group in the same bank), so
        # each (parity, gate) gets its own bank: seed(t) -> W(t) ->
        # seed(t+2) -> ... per bank.
        zfull = psZ.tile([U, 8, 512], F32, tag="z", name="zfull")
        zq = [zfull[:, bass.ds(4 * p, 4), 0:B] for p in range(2)]

        def seed_step(k, t, parity):
            xs = xsb[k % 2][:, :, t]  # [F+1, B]
            for g in range(4):
                nc.tensor.matmul(
                    zfull[:, 4 * parity + g, 0:B],
                    lhsT=Kb_sb[:, g * U : (g + 1) * U],
                    rhs=xs,
                    start=True,
                    stop=False,
                    skip_group_check=True,
                )

        # prologue: seed step 0
        seed_step(0, 0, 0)

        def emit_body(wrap_seed):
            nonlocal step
            for k in range(n_chunks):
                if k + 1 < n_chunks:
                    nc.sync.dma_start(
                        xsb[(k + 1) % 2], xT_view[:, :, bass.ds((k + 1) * CH, CH)]
                    )
                for t in range(CH):
                    p = step % 2
                    c_old = cbuf[step % 2]
                    c_new = cbuf[(step + 1) % 2]
                    # ---- PE: z[:, g, :] += W_g^T @ h ----
                    for g in range(4):
                        nc.tensor.matmul(
                            zfull[:, 4 * p + g, 0:B],
                            lhsT=W_sb[:, g * U : (g + 1) * U],
                            rhs=hT,
                            start=(tvar == "noseed" and g == 0),
                            stop=True,
                            skip_group_check=True,
                        )
                    # seed matmuls for the next step ride the PE gap
                    last_step = k == n_chunks - 1 and t == CH - 1
                    if tvar == "base":
                        if not last_step:
                            nk, nt = (k, t + 1) if t + 1 < CH else (k + 1, 0)
                            seed_step(nk, nt, 1 - p)
                        elif wrap_seed:
                            seed_step(0, 0, 1 - p)
                    # ---- ACT: one sigmoid over all 4 gate blocks ----
                    z_t = zq[p]
                    sg = gates.tile([U, 4, B], F32, tag="sg", name=f"sg_{step}")
                    nc.scalar.activation(sg[:], z_t, func=AF.Sigmoid)
                    # ---- fc = f * c_old (off critical path) ----
                    fc = gates.tile([U, B], F32, tag="fc", name=f"fc_{step}")
                    fc_eng = nc.gpsimd if fc_pool else nc.vector
                    fc_eng.tensor_mul(fc, sg[:, G_F, :], c_old)
                    # ---- DVE: v = (s_g - 0.5)*i; c = 2*v + fc ----
                    # (2*(s_g-0.5)*i = i*(2*sigmoid(2 z_g)-1) = i*tanh(z_g))
                    v_t = gates.tile([U, B], F32, tag="v", name=f"v_{step}")
                    nc.vector.scalar_tensor_tensor(
                        out=v_t, in0=sg[:, G_G, :], scalar=0.5,
                        in1=sg[:, G_I, :], op0=ALU.subtract, op1=ALU.mult,
                    )
                    nc.vector.scalar_tensor_tensor(
                        out=c_new, in0=v_t, scalar=2.0, in1=fc,
                        op0=ALU.mult, op1=ALU.add,
                    )
                    # ---- ACT: th = tanh(c); DVE: h = o * th (bf16) ----
                    th = gates.tile([U, B], F32, tag="th", name=f"th_{step}")
                    nc.scalar.activation(th, c_new, func=AF.Tanh)
                    nc.vector.tensor_mul(hT, sg[:, G_O, :], th)
                    step += 1

        step = 0  # global step counter (c / z ping-pong parity)
        if n_loop == 1:
            emit_body(wrap_seed=False)
        else:
            assert T % 2 == 0 and (B * T) % 2 == 0
            with tc.For_i(0, n_loop, 1):
                emit_body(wrap_seed=True)
                step = 0

        nc.sync.dma_start(out_d[:], hT)

    nc.finalize()
    return nc


def _prep_inputs(x, kernel, recurrent_kernel, bias, T):
    """Host-side reordering. Returns per-core input maps.

    Gate blocks reordered [i, f, o, g]; the g block is scaled by 2 so the
    device's single sigmoid gives tanh via 2*sigmoid(2z) - 1. Bias is
    stacked under kernel as a ones-row contraction.
    """
    perm = np.concatenate([np.arange(g * U, (g + 1) * U) for g in GATE_PERM])
    scale = np.ones((4 * U,), np.float32)
    scale[3 * U :] = 2.0  # g block (device order) pre-doubled
    w_np = np.ascontiguousarray(
        recurrent_kernel[:, perm] * scale, dtype=np.float32
    )
    kern_p = kernel[:, perm] * scale
    bias_p = bias[perm] * scale
    import ml_dtypes

    kb_np = np.concatenate(
        [kern_p, bias_p[None, :]], axis=0
    ).astype(ml_dtypes.bfloat16)  # [F+1, 4U]
    in_maps = []
    for c in range(N_CORES):
        xs = x[c * B : (c + 1) * B]  # [B, T, F]
        xT = np.empty((F + 1, B * T), dtype=np.float32)
        xT[:F] = xs.transpose(2, 0, 1).reshape(F, B * T)
        xT[F] = 1.0
        xT = xT.astype(ml_dtypes.bfloat16)
        in_maps.append({"xT": xT, "w": w_np, "kb": kb_np})
    return in_maps


def run_lstm(x, kernel, recurrent_kernel, bias, T=T_FULL, CH=512, trace=False,
             **_compat):
    nc = build_nc(T, CH)
    in_maps = _prep_inputs(x, kernel, recurrent_kernel, bias, T)
    res = run_bass_kernel_spmd(
        nc, in_maps, core_ids=list(range(N_CORES)), trace=trace
    )
    h = np.zeros((N_CORES * B, U), dtype=np.float32)
    for c in range(N_CORES):
        h[c * B : (c + 1) * B] = res.results[c]["hT_out"].astype(np.float32).T
    return h, res


def kernel(x, kernel, recurrent_kernel, bias):
    x = np.asarray(x)
    kernel = np.asarray(kernel)
    recurrent_kernel = np.asarray(recurrent_kernel)
    bias = np.asarray(bias)
    h, _ = run_lstm(x, kernel, recurrent_kernel, bias)
    return h


# revision 23
# speedup vs baseline: 1.4158x; 1.4158x over previous
"""Trainium2 Bass kernel for CarbonAwareLSTM.

B=64, T=4096, F=64, U=128. Keras LSTM (gate order i,f,c,o), returns last
hidden state h_T [B, U].

Strategy (data-parallel over batch, 8 cores x 8 rows). The recurrence is
latency-bound (per-step serial chain across engines, ~1.85 us/step on
HW), so the design minimizes instructions on the per-step critical
cycle:

- Input projection z_seed = [K; bias]^T @ [x; 1] (bf16) is matmul'd
  directly into the per-step PSUM z tiles (no xw SBUF buffer, no
  evacuation instructions). The 4 seed matmuls for step t+1 are emitted
  right after step t's W matmuls so they ride PE idle gaps; bf16
  stationaries keep their Ldweights cost negligible (fp32 seeds
  measured +575 ns/step on HW).
- PSUM accumulation on HW only tolerates ONE open
  start->accumulate->stop group per 2KB bank (a start=True corrupts any
  other open group in its bank), so gate g of step-parity p owns bank
  4p+g of a single [U, 8, 512] PSUM tile: per bank, seed(t) -> W(t) ->
  seed(t+2) is strictly sequential. Wide multi-step seed groups also
  corrupt earlier columns and cannot be used.
- Per step: 4 accumulating W matmuls (bf16) onto the seeded PSUM; ONE
  sigmoid over all 4 gate blocks [U, 4, B] (the g gate's z is
  pre-scaled by 2 host-side so tanh(z) = 2*sigmoid(2z) - 1); DVE:
  v = (s_g - 0.5)*i then c = 2v + fc (scalar_tensor_tensor, two fused
  ops replace three); Pool (gpsimd) computes fc = f*c_old off the
  critical path; ACT tanh(c); DVE h = o*tanh(c) written directly as
  bf16 (stationary operand of the next step's W matmuls).
- c is double-buffered in SBUF (ping-pong) so Pool's read of c_old
  never serializes against the c_new write; h/th/sg all stay in SBUF
  (DVE pays 2x access latency on PSUM operands; ACT's PSUM preference
  never wins once its consumers' penalties are counted).
"""

import sys

sys.path.insert(0, "/opt/trn_rl_repo")

from contextlib import ExitStack

import numpy as np

import concourse.bacc as bacc
import concourse.bass as bass
import concourse.tile as tile
from concourse import mybir
from concourse.bass_utils import run_bass_kernel_spmd

B_TOTAL = 64
T_FULL = 4096
F = 64
U = 128
N_CORES = 8
B = B_TOTAL // N_CORES  # batch rows per core

F32 = mybir.dt.float32
BF16 = mybir.dt.bfloat16
AF = mybir.ActivationFunctionType
ALU = mybir.AluOpType

# gate block order used on device: [i, f, o, g]; reference order is [i, f, g, o]
GATE_PERM = [0, 1, 3, 2]
G_I, G_F, G_O, G_G = 0, 1, 2, 3


def build_nc(T: int, CH: int = 512, fc_pool: bool = False, n_loop: int = 1,
             tvar: str = "base", **_compat) -> bass.Bass:
    """Build the single-core Bass program (run SPMD on 8 cores).

    CH = timesteps per DMA chunk. fc_pool: compute fc = f*c_old on the
    Pool (gpsimd) engine instead of DVE (measured slower on HW: real Q7
    launch overhead exceeds the cost model's 95ns). n_loop > 1 repeats the whole
    recurrence in a hardware loop (timing builds only). tvar selects
    timing-only structural variants (numerically wrong except "base"):
      noseed       - no seed matmuls (W-mm g0 start=True)
      sigma_contig - sigma reads a contiguous single-bank window
      onegroup     - v1-style single-bank z group, no seeds, contig sigma
    """
    assert T % CH == 0
    n_chunks = T // CH

    nc = bacc.Bacc(None, target_bir_lowering=False, debug=False)

    # xT: [F+1, B, T] with a trailing ones row (bias via contraction)
    xT_d = nc.dram_tensor("xT", [F + 1, B * T], BF16, kind="ExternalInput")
    w_d = nc.dram_tensor("w", [U, 4 * U], F32, kind="ExternalInput")
    kb_d = nc.dram_tensor("kb", [F + 1, 4 * U], BF16, kind="ExternalInput")
    out_d = nc.dram_tensor("hT_out", [U, B], BF16, kind="ExternalOutput")

    with tile.TileContext(nc) as tc, ExitStack() as ctx:
        singles = ctx.enter_context(tc.tile_pool(name="singles", bufs=1))
        xsb_pool = ctx.enter_context(tc.tile_pool(name="xsb", bufs=2))
        psZ = ctx.enter_context(tc.tile_pool(name="psZ", bufs=1, space="PSUM"))
        gates = ctx.enter_context(tc.tile_pool(name="gates", bufs=2))

        W_f32 = singles.tile([U, 4 * U], F32)
        nc.sync.dma_start(W_f32, w_d[:])
        W_sb = singles.tile([U, 4 * U], BF16)
        nc.vector.tensor_copy(W_sb, W_f32)
        Kb_sb = singles.tile([F + 1, 4 * U], BF16)
        nc.sync.dma_start(Kb_sb, kb_d[:])

        hT = singles.tile([U, B], BF16, tag="hT", name="hT")
        nc.vector.memset(hT, 0.0)
        cbuf = [
            singles.tile([U, B], F32, tag=f"c{p}", name=f"c{p}") for p in range(2)
        ]
        nc.vector.memset(cbuf[0], 0.0)

        xT_view = xT_d[:].rearrange("f (b t) -> f b t", b=B)

        # chunk-level SBUF x buffers (DMA'd one chunk ahead)
        xsb = []
        for k in range(min(n_chunks, 2)):
            t_ = xsb_pool.tile([F + 1, B, CH], BF16, tag="xsb", name=f"xsb{k}")
            xsb.append(t_)
        nc.sync.dma_start(xsb[0], xT_view[:, :, bass.ds(0, CH)])

        # z PSUM: one tile spanning all 8 banks; gate g of step-parity p
        # owns bank 4p+g (cols 0:B). Hardware PSUM accumulation requires
        # per-bank groups to be small and strictly sequential (a
        # start=True corrupts any other open group in the same bank), so
        # each (parity, gate) gets its own bank: seed(t) -> W(t) ->
        # seed(t+2) -> ... per bank.
        zfull = psZ.tile([U, 8, 512], F32, tag="z", name="zfull")
        zq = [zfull[:, bass.ds(4 * p, 4), 0:B] for p in range(2)]

        def seed_step(k, t, parity):
            xs = xsb[k % 2][:, :, t]  # [F+1, B]
            for g in range(4):
                nc.tensor.matmul(
                    zfull[:, 4 * parity + g, 0:B],
                    lhsT=Kb_sb[:, g * U : (g + 1) * U],
                    rhs=xs,
                    start=True,
                    stop=False,
                    skip_group_check=True,
                )

        # prologue: seed step 0
        seed_step(0, 0, 0)

        def emit_body(wrap_seed):
            nonlocal step
            for k in range(n_chunks):
                if k + 1 < n_chunks:
                    nc.sync.dma_start(
                        xsb[(k + 1) % 2], xT_view[:, :, bass.ds((k + 1) * CH, CH)]
                    )
                for t in range(CH):
                    p = step % 2
                    c_old = cbuf[step % 2]
                    c_new = cbuf[(step + 1) % 2]
                    # ---- PE: z[:, g, :] += W_g^T @ h ----
                    for g in range(4):
                        nc.tensor.matmul(
                            zfull[:, 4 * p + g, 0:B],
                            lhsT=W_sb[:, g * U : (g + 1) * U],
                            rhs=hT,
                            start=(tvar == "noseed" and g == 0),
                            stop=True,
                            skip_group_check=True,
                        )
                    # seed matmuls for the next step ride the PE gap
                    last_step = k == n_chunks - 1 and t == CH - 1
                    if tvar == "base":
                        if not last_step:
                            nk, nt = (k, t + 1) if t + 1 < CH else (k + 1, 0)
                            seed_step(nk, nt, 1 - p)
                        elif wrap_seed:
                            seed_step(0, 0, 1 - p)
                    # ---- ACT: one sigmoid over all 4 gate blocks ----
                    z_t = zq[p]
                    sg = gates.tile([U, 4, B], F32, tag="sg", name=f"sg_{step}")
                    nc.scalar.activation(sg[:], z_t, func=AF.Sigmoid)
                    # ---- fc = f * c_old (off critical path) ----
                    fc = gates.tile([U, B], F32, tag="fc", name=f"fc_{step}")
                    fc_eng = nc.gpsimd if fc_pool else nc.vector
                    fc_eng.tensor_mul(fc, sg[:, G_F, :], c_old)
                    # ---- DVE: v = (s_g - 0.5)*i; c = 2*v + fc ----
                    # (2*(s_g-0.5)*i = i*(2*sigmoid(2 z_g)-1) = i*tanh(z_g))
                    v_t = gates.tile([U, B], F32, tag="v", name=f"v_{step}")
                    nc.vector.scalar_tensor_tensor(
                        out=v_t, in0=sg[:, G_G, :], scalar=0.5,
                        in1=sg[:, G_I, :], op0=ALU.subtract, op1=ALU.mult,
                    )
                    nc.vector.scalar_tensor_tensor(
                        out=c_new, in0=v_t, scalar=2.0, in1=fc,
                        op0=ALU.mult, op1=ALU.add,
                    )
                    # ---- ACT: th = tanh(c); DVE: h = o * th (bf16) ----
                    th = gates.tile([U, B], F32, tag="th", name=f"th_{step}")
                    nc.scalar.activation(th, c_new, func=AF.Tanh)
                    nc.vector.tensor_mul(hT, sg[:, G_O, :], th)
                    step += 1

        step = 0  # global step counter (c / z ping-pong parity)
        if n_loop == 1:
            emit_body(wrap_seed=False)
        else:
            assert T % 2 == 0 and (B * T) % 2 == 0
            with tc.For_i(0, n_loop, 1):
                emit_body(wrap_seed=True)
                step = 0

        nc.sync.dma_start(out_d[:], hT)

    nc.finalize()
    return nc


def _prep_inputs(x, kernel, recurrent_kernel, bias, T):
    """Host-side reordering. Returns per-core input maps.

    Gate blocks reordered [i, f, o, g]; the g block is scaled by 2 so the
    device's single sigmoid gives tanh via 2*sigmoid(2z) - 1. Bias is
    stacked under kernel as a ones-row contraction.
    """
    perm = np.concatenate([np.arange(g * U, (g + 1) * U) for g in GATE_PERM])
    scale = np.ones((4 * U,), np.float32)
    scale[3 * U :] = 2.0  # g block (device order) pre-doubled
    w_np = np.ascontiguousarray(
        recurrent_kernel[:, perm] * scale, dtype=np.float32
    )
    kern_p = kernel[:, perm] * scale
    bias_p = bias[perm] * scale
    import ml_dtypes

    kb_np = np.concatenate(
        [kern_p, bias_p[None, :]], axis=0
    ).astype(ml_dtypes.bfloat16)  # [F+1, 4U]
    in_maps = []
    for c in range(N_CORES):
        xs = x[c * B : (c + 1) * B]  # [B, T, F]
        xT = np.empty((F + 1, B * T), dtype=np.float32)
        xT[:F] = xs.transpose(2, 0, 1).reshape(F, B * T)
        xT[F] = 1.0
        xT = xT.astype(ml_dtypes.bfloat16)
        in_maps.append({"xT": xT, "w": w_np, "kb": kb_np})
    return in_maps


def run_lstm(x, kernel, recurrent_kernel, bias, T=T_FULL, CH=512, trace=False,
             **_compat):
    nc = build_nc(T, CH)
    in_maps = _prep_inputs(x, kernel, recurrent_kernel, bias, T)
    res = run_bass_kernel_spmd(
        nc, in_maps, core_ids=list(range(N_CORES)), trace=trace
    )
    h = np.zeros((N_CORES * B, U), dtype=np.float32)
    for c in range(N_CORES):
        h[c * B : (c + 1) * B] = res.results[c]["hT_out"].astype(np.float32).T
    return h, res


def kernel(x, kernel, recurrent_kernel, bias):
    x = np.asarray(x)
    kernel = np.asarray(kernel)
    recurrent_kernel = np.asarray(recurrent_kernel)
    bias = np.asarray(bias)
    h, _ = run_lstm(x, kernel, recurrent_kernel, bias)
    return h
